# revision 35
# baseline (speedup 1.0000x reference)
"""Multi-head attention (B=4, S=2048, D=1024, H=16) on 8 Trainium2 cores.

Sharding: core c handles batch b = c//2 and head-group g = c%2 (8 heads,
512 features). Device program is identical on all cores (SPMD); the host
feeds each core its batch's activations (pre-transposed to [D, S]) and its
head-group's weight slices, and sums the two partial output projections
per batch at the end (core g=1 gets a zero bias so the bias is added once).

Device-side layout (per core):
  qT/kT:  [512 f, S]  (f on partitions, chunked [128, 4, S])   f = 8 heads x 64
  v:      [S, 520]    (kj on partitions, per head 64 cols + a ones column)
  scores: sT[kj, qi] = k q^T tiles in PSUM -> exp on ACT -> probsT bf16 SBUF
  PV:     out_aug[65, qi] = v_aug^T @ probsT  (row 64 = softmax denominator,
          via the ones column), accumulated over kj chunks in PSUM
  divide: broadcast denominator row over 64 partitions (DRAM bounce DMA),
          approx-reciprocal + multiply on DVE
  y:      yT[j, qi] partial = woT^T @ outT (+ bias), f32 to DRAM

PSUM budget is static (8 banks): 3 x [128,1024] score/projection tiles (tag
"wide") + 1 x [65,1024] PV accumulator, so the whole body can also sit
inside a hardware repeat loop (`repeat` > 1, timing harness only). The
accumulator is single-buffered but released immediately after the last PV
via one DVE copy to SBUF; triple-buffered score tiles keep the exp stream
gapless (ACT measures 100% busy through the attention region).

Why this shape: the kernel is ACT-bound, not PE-bound. The softmax exp must
stream all 33.5M score elements per core through the scalar engine at
1 elem/lane/cycle (~220us), while PE matmul work is ~330us busy but attention
matmuls only reach 50% array utilization (contraction/output dim = 64 < 128).
The exp instruction width (1024) is the PSUM-bank-budget optimum.
Interleaving projection work into the attention stream was tried and measures
worse (PSUM slot FIFO rotation stalls the exp pipeline), so phases stay
serial: projections (PE ~100%) -> attention (ACT 100%) -> output projection.
The first weight/activation loads are split per contraction chunk so the
first matmul starts ~1us in instead of waiting a 2MB transfer.
"""

import numpy as np
import ml_dtypes

import concourse.bacc as bacc
import concourse.bass as bass
import concourse.mybir as mybir
import concourse.tile as tile

BF16 = mybir.dt.bfloat16
F32 = mybir.dt.float32

B, S, D, H = 4, 2048, 1024, 16
HD = 64
N_CORES = 8
F = D // 2  # features per core (8 heads x 64)


def build_nc(s=S, d=D, f=F, num_devices=N_CORES, repeat=1):
    """Build the per-core Bass program. Parametrized so a small config can be
    validated in CoreSim; the shipped kernel uses the defaults."""
    hpc = f // HD          # heads per core
    dc = d // 128          # contraction chunks for projections
    fc = f // 128          # feature chunks (out partitions for q/k proj)
    jc = d // 128          # output-feature chunks for the final projection
    kc = s // 128          # kj chunks for attention
    aq = 1024 if s % 1024 == 0 else 512  # wide tile / attention qi block
    nq = s // aq
    scale = 1.0 / np.sqrt(HD)
    assert f <= 512

    nc = bacc.Bacc("TRN2", target_bir_lowering=False, debug=False,
                   num_devices=num_devices)

    xq = nc.dram_tensor("xq_t", [d, s], BF16, kind="ExternalInput").ap()
    xk = nc.dram_tensor("xk_t", [d, s], BF16, kind="ExternalInput").ap()
    xv = nc.dram_tensor("xv_t", [d, s], BF16, kind="ExternalInput").ap()
    wq = nc.dram_tensor("wq_t", [d, f], BF16, kind="ExternalInput").ap()
    wk = nc.dram_tensor("wk_t", [d, f], BF16, kind="ExternalInput").ap()
    wv = nc.dram_tensor("wv_t", [d, f], BF16, kind="ExternalInput").ap()
    wo = nc.dram_tensor("wo_t", [f, d], BF16, kind="ExternalInput").ap()
    bo = nc.dram_tensor("bo_r", [128, jc], F32, kind="ExternalInput").ap()
    y = nc.dram_tensor("y_t", [d, s], F32, kind="ExternalOutput").ap()

    with tile.TileContext(nc) as tc:
        with (
            tc.tile_pool(name="weights", bufs=1) as wpool,
            tc.tile_pool(name="store", bufs=1) as store,
            tc.tile_pool(name="xin", bufs=2) as xpool,
            tc.tile_pool(name="probs", bufs=3) as ppool,
            tc.tile_pool(name="bcast", bufs=2) as bpool,
            tc.tile_pool(name="odiv", bufs=3) as opool,
            tc.tile_pool(name="ystage", bufs=3) as ypool,
            tc.tile_pool(name="ldram", bufs=4, space="DRAM") as dpool,
            tc.tile_pool(name="psum", bufs=3, space="PSUM") as psum,
            tc.tile_pool(name="psumo", bufs=1, space="PSUM") as psumo,
        ):
            # ---- persistent SBUF state ----
            wq_sb = wpool.tile([128, dc, f], BF16, tag="wq")
            wk_sb = wpool.tile([128, dc, f], BF16, tag="wk")
            wv_sb = wpool.tile([128, dc, f], BF16, tag="wv")
            wo_sb = wpool.tile([128, fc, d], BF16, tag="wo")
            bo_sb = wpool.tile([128, jc], F32, tag="bo")
            for ci in range(dc):
                nc.sync.dma_start(
                    out=wk_sb[:, ci],
                    in_=wk.rearrange("(c p) f -> p c f", p=128)[:, ci])
            nc.sync.dma_start(out=wq_sb, in_=wq.rearrange("(c p) f -> p c f", p=128))
            nc.sync.dma_start(out=wv_sb, in_=wv.rearrange("(c p) f -> p c f", p=128))
            nc.sync.dma_start(out=wo_sb, in_=wo.rearrange("(c p) j -> p c j", p=128))
            nc.sync.dma_start(out=bo_sb, in_=bo)

            qT_sb = store.tile([128, fc, s], BF16, tag="qT")
            kT_sb = store.tile([128, fc, s], BF16, tag="kT")
            v_sb = store.tile([128, kc, hpc * 65], BF16, tag="v")
            outT_sb = store.tile([128, fc, s], BF16, tag="outT")

            # x slices are loaded once and shared between the prefix (feature
            # chunk 0) and the deferred per-chunk projections; the pool slot
            # stays live until the last emitted consumer
            xcache = {}

            def get_x(x_dram, q):
                key = (x_dram.tensor.name, q)
                if key not in xcache:
                    xr = x_dram.rearrange("(c p) s -> p c s", p=128)
                    t = xpool.tile([128, dc, aq], BF16, tag="x")
                    for ci in range(dc):
                        nc.sync.dma_start(
                            out=t[:, ci], in_=xr[:, ci, q * aq:(q + 1) * aq])
                    xcache[key] = t
                return xcache[key]

            def qk_proj_slice(x_dram, w_sb, dst, q, fis):
                """qT/kT projection for one aq slice and a list of f chunks."""
                sl = slice(q * aq, (q + 1) * aq)
                x_sb = get_x(x_dram, q)
                for fi in fis:
                    ps = psum.tile([128, aq], F32, tag="wide")
                    for qq in range(aq // 512):
                        for ci in range(dc):
                            nc.tensor.matmul(
                                ps[:, qq * 512:(qq + 1) * 512],
                                lhsT=w_sb[:, ci, fi * 128:(fi + 1) * 128],
                                rhs=x_sb[:, ci, qq * 512:(qq + 1) * 512],
                                start=(ci == 0), stop=(ci == dc - 1))
                    nc.vector.tensor_copy(out=dst[:, fi, sl], in_=ps)

            def v_proj_slice(q):
                """v projection (with ones columns interleaved) for one slice."""
                xr = xv.rearrange("(c p) s -> p c s", p=128)
                nchunks = aq // 128   # kj chunks per aq slice
                cpt = aq // f         # kj chunks packed per psum tile
                sl = slice(q * aq, (q + 1) * aq)
                x_sb = xpool.tile([128, dc, aq], BF16, tag="x")
                nc.sync.dma_start(out=x_sb, in_=xr[:, :, sl])
                for t in range(nchunks // cpt):
                    ps = psum.tile([128, aq], F32, tag="wide")
                    for k4 in range(cpt):
                        ck = t * cpt + k4
                        for ci in range(dc):
                            nc.tensor.matmul(
                                ps[:, k4 * f:(k4 + 1) * f],
                                lhsT=x_sb[:, ci, ck * 128:(ck + 1) * 128],
                                rhs=wv_sb[:, ci],
                                start=(ci == 0), stop=(ci == dc - 1))
                        cix = q * nchunks + ck
                        nc.vector.tensor_copy(
                            out=v_sb[:, cix].rearrange(
                                "p (h x) -> p h x", x=65)[:, :, 0:64],
                            in_=ps[:, k4 * f:(k4 + 1) * f].rearrange(
                                "p (h x) -> p h x", x=64))

            def attention(h, qb, queue=()):
                hp = (h % 2) * 64
                hc = h // 2
                kTh = kT_sb[hp:hp + 64, hc]
                qTh = qT_sb[hp:hp + 64, hc]
                oaug = psumo.tile([65, aq], F32, tag="oaug")
                for c in range(kc):
                    if queue and c in (kc // 3, (2 * kc) // 3):
                        queue.pop(0)()
                    sc = psum.tile([128, aq], F32, tag="wide")
                    for qq in range(aq // 512):
                        nc.tensor.matmul(
                            sc[:, qq * 512:(qq + 1) * 512],
                            lhsT=kTh[:, c * 128:(c + 1) * 128],
                            rhs=qTh[:, qb * aq + qq * 512:
                                    qb * aq + (qq + 1) * 512],
                            start=True, stop=True)
                    pr = ppool.tile([128, aq], BF16, tag="pr")
                    nc.scalar.activation(
                        out=pr, in_=sc,
                        func=mybir.ActivationFunctionType.Exp,
                        scale=float(scale))
                    for qq in range(aq // 512):
                        nc.tensor.matmul(
                            oaug[:, qq * 512:(qq + 1) * 512],
                            lhsT=v_sb[:, c, h * 65:(h + 1) * 65],
                            rhs=pr[:, qq * 512:(qq + 1) * 512],
                            start=(c == 0), stop=(c == kc - 1),
                            skip_group_check=True)
                # stage the denominator row to SBUF (DMA cannot read PSUM),
                # broadcast it over 64 partitions via a DRAM bounce, divide
                ls = bpool.tile([65, aq], F32, tag="ls")
                nc.vector.tensor_copy(out=ls, in_=oaug)
                ld = dpool.tile([1, aq], F32, tag="ld")
                nc.sync.dma_start(out=ld, in_=ls[64:65])
                bc = bpool.tile([64, aq], F32, tag="bc")
                nc.sync.dma_start(out=bc, in_=ld.to_broadcast([64, aq]))
                nc.vector.reciprocal_approx_fast(out=bc, in_=bc)
                od = opool.tile([64, aq], BF16, tag="od")
                nc.vector.tensor_mul(out=od, in0=ls[0:64], in1=bc)
                nc.sync.dma_start(
                    out=outT_sb[hp:hp + 64, hc, qb * aq:(qb + 1) * aq],
                    in_=od)

            def wo_proj_group(q, j):
                """output projection yT[j chunk, aq slice] = woT^T @ outT + bias"""
                sl = slice(q * aq, (q + 1) * aq)
                ps = psum.tile([128, aq], F32, tag="wide")
                for qq in range(aq // 512):
                    for fi in range(fc):
                        nc.tensor.matmul(
                            ps[:, qq * 512:(qq + 1) * 512],
                            lhsT=wo_sb[:, fi, j * 128:(j + 1) * 128],
                            rhs=outT_sb[:, fi,
                                        q * aq + qq * 512:q * aq + (qq + 1) * 512],
                            start=(fi == 0), stop=(fi == fc - 1))
                ys = ypool.tile([128, aq], F32, tag="ys")
                nc.scalar.activation(
                    out=ys, in_=ps,
                    func=mybir.ActivationFunctionType.Identity,
                    bias=bo_sb[:, j:j + 1], scale=1.0)
                nc.sync.dma_start(out=y[j * 128:(j + 1) * 128, sl], in_=ys)

            def body(_iv=None):
                # ones columns of v_aug (one strided memset per kj chunk)
                for c in range(kc):
                    nc.vector.memset(
                        v_sb[:, c].rearrange("p (h x) -> p h x", x=65)[:, :, 64:65],
                        1.0)

                # serial phases: projections, attention, output projection
                for q in range(nq):
                    qk_proj_slice(xk, wk_sb, kT_sb, q, range(fc))
                for q in range(nq):
                    qk_proj_slice(xq, wq_sb, qT_sb, q, range(fc))
                for q in range(nq):
                    v_proj_slice(q)
                for h in range(hpc):
                    for qb in range(nq):
                        attention(h, qb)
                for q in range(nq):
                    for j in range(jc):
                        wo_proj_group(q, j)

            if repeat == 1:
                body()
            else:
                with tc.For_i(0, repeat, 1) as iv:
                    body(iv)

    nc.compile()
    return nc


def make_core_inputs(query, key, value, wq, wk, wv, wo, bo):
    """Host-side sharding: per-core input dicts (bf16 casts + transposes)."""
    bf = ml_dtypes.bfloat16
    query, key, value = (np.asarray(t, np.float32) for t in (query, key, value))
    wq, wk, wv, wo, bo = (np.asarray(t, np.float32) for t in (wq, wk, wv, wo, bo))
    ins = []
    for c in range(N_CORES):
        b, g = c // 2, c % 2
        fs = slice(g * F, (g + 1) * F)
        ins.append({
            "xq_t": np.ascontiguousarray(query[b].astype(bf).T),
            "xk_t": np.ascontiguousarray(key[b].astype(bf).T),
            "xv_t": np.ascontiguousarray(value[b].astype(bf).T),
            "wq_t": np.ascontiguousarray(wq[fs, :].T.astype(bf)),
            "wk_t": np.ascontiguousarray(wk[fs, :].T.astype(bf)),
            "wv_t": np.ascontiguousarray(wv[fs, :].T.astype(bf)),
            "wo_t": np.ascontiguousarray(wo[:, fs].T.astype(bf)),
            "bo_r": (bo.reshape(D // 128, 128).T.astype(np.float32)
                     if g == 0 else np.zeros((128, D // 128), np.float32)),
        })
    return ins


_NC_CACHE = None


def kernel(query, key, value, wq, wk, wv, wo, bo):
    global _NC_CACHE
    from concourse.bass_utils import run_bass_kernel_spmd

    if _NC_CACHE is None:
        _NC_CACHE = build_nc()
    ins = make_core_inputs(query, key, value, wq, wk, wv, wo, bo)
    res = run_bass_kernel_spmd(_NC_CACHE, ins, list(range(N_CORES)))
    out = np.empty((B, S, D), np.float32)
    for b in range(B):
        out[b] = (res.results[2 * b]["y_t"] + res.results[2 * b + 1]["y_t"]).T
    return out


# revision 43
# speedup vs baseline: 1.0529x; 1.0529x over previous
"""Multi-head attention (B=4, S=2048, D=1024, H=16) on 8 Trainium2 cores.

Sharding: core c handles batch b = c//2 and head-group g = c%2 (8 heads,
512 features). Device program is identical on all cores (SPMD); the host
feeds each core its batch's activations (pre-transposed to [D, S]) and its
head-group's weight slices, and sums the two partial output projections
per batch at the end (core g=1 gets a zero bias so the bias is added once).

Device-side layout (per core):
  qT/kT:  [512 f, S]  (f on partitions, chunked [128, 4, S])   f = 8 heads x 64
  v:      [S, 520]    (kj on partitions, per head 64 cols + a ones column)
  scores: sT[kj, qi] = k q^T tiles in PSUM -> exp on ACT -> probsT bf16 SBUF
  PV:     out_aug[65, qi] = v_aug^T @ probsT  (row 64 = softmax denominator,
          via the ones column), accumulated over kj chunks in PSUM
  divide: broadcast denominator row over 64 partitions (DRAM bounce DMA),
          approx-reciprocal + multiply on DVE
  y:      yT[j, qi] partial = woT^T @ outT (+ bias), f32 to DRAM

PSUM budget is static (8 banks): 3 x [128,1024] score/projection tiles (tag
"wide") + 1 x [65,1024] PV accumulator, so the whole body can also sit
inside a hardware repeat loop (`repeat` > 1, timing harness only). The
accumulator is single-buffered but released immediately after the last PV
via one DVE copy to SBUF; triple-buffered score tiles keep the exp stream
gapless (ACT measures 100% busy through the attention region).

Why this shape: the kernel is ACT-bound, not PE-bound. The softmax exp must
stream all 33.5M score elements per core through the scalar engine at
1 elem/lane/cycle (~220us), while PE matmul work is ~330us busy but attention
matmuls only reach 50% array utilization (contraction/output dim = 64 < 128).
The exp instruction width (1024) is the PSUM-bank-budget optimum.
Interleaving projection work into the attention stream was tried and measures
worse (PSUM slot FIFO rotation stalls the exp pipeline), so phases stay
serial: projections (PE ~100%) -> attention (ACT 100%) -> output projection.
The first weight/activation loads are split per contraction chunk so the
first matmul starts ~1us in instead of waiting a 2MB transfer.
"""

import numpy as np
import ml_dtypes

import concourse.bacc as bacc
import concourse.bass as bass
import concourse.mybir as mybir
import concourse.tile as tile

BF16 = mybir.dt.bfloat16
F32 = mybir.dt.float32

B, S, D, H = 4, 2048, 1024, 16
HD = 64
N_CORES = 8
F = D // 2  # features per core (8 heads x 64)


def build_nc(s=S, d=D, f=F, num_devices=N_CORES, repeat=1):
    """Build the per-core Bass program. Parametrized so a small config can be
    validated in CoreSim; the shipped kernel uses the defaults."""
    hpc = f // HD          # heads per core
    dc = d // 128          # contraction chunks for projections
    fc = f // 128          # feature chunks (out partitions for q/k proj)
    jc = d // 128          # output-feature chunks for the final projection
    kc = s // 128          # kj chunks for attention
    aq = 1024 if s % 1024 == 0 else 512  # wide tile / attention qi block
    nq = s // aq
    scale = 1.0 / np.sqrt(HD)
    assert f <= 512

    nc = bacc.Bacc("TRN2", target_bir_lowering=False, debug=False,
                   num_devices=num_devices)

    xq = nc.dram_tensor("xq_t", [d, s], BF16, kind="ExternalInput").ap()
    xk = nc.dram_tensor("xk_t", [d, s], BF16, kind="ExternalInput").ap()
    xv = nc.dram_tensor("xv_t", [d, s], BF16, kind="ExternalInput").ap()
    wq = nc.dram_tensor("wq_t", [d, f], BF16, kind="ExternalInput").ap()
    wk = nc.dram_tensor("wk_t", [d, f], BF16, kind="ExternalInput").ap()
    wv = nc.dram_tensor("wv_t", [d, f], BF16, kind="ExternalInput").ap()
    wo = nc.dram_tensor("wo_t", [f, d], BF16, kind="ExternalInput").ap()
    bo = nc.dram_tensor("bo_r", [128, jc], F32, kind="ExternalInput").ap()
    y = nc.dram_tensor("y_t", [d, s], F32, kind="ExternalOutput").ap()

    with tile.TileContext(nc) as tc:
        with (
            tc.tile_pool(name="weights", bufs=1) as wpool,
            tc.tile_pool(name="store", bufs=1) as store,
            tc.tile_pool(name="xin", bufs=2) as xpool,
            tc.tile_pool(name="probs", bufs=3) as ppool,
            tc.tile_pool(name="bcast", bufs=2) as bpool,
            tc.tile_pool(name="odiv", bufs=3) as opool,
            tc.tile_pool(name="ystage", bufs=3) as ypool,
            tc.tile_pool(name="ldram", bufs=4, space="DRAM") as dpool,
            tc.tile_pool(name="psum", bufs=3, space="PSUM") as psum,
            tc.tile_pool(name="psumo", bufs=1, space="PSUM") as psumo,
        ):
            # ---- persistent SBUF state ----
            wq_sb = wpool.tile([128, dc, f], BF16, tag="wq")
            wk_sb = wpool.tile([128, dc, f], BF16, tag="wk")
            wv_sb = wpool.tile([128, dc, f], BF16, tag="wv")
            wo_sb = wpool.tile([128, fc, d], BF16, tag="wo")
            bo_sb = wpool.tile([128, jc], F32, tag="bo")
            nc.sync.dma_start(out=wk_sb, in_=wk.rearrange("(c p) f -> p c f", p=128))
            # bulk weights ride the gpsimd DMA queue so the first x-slice
            # chunks on the sync queue aren't stuck behind them at startup
            nc.gpsimd.dma_start(out=wq_sb, in_=wq.rearrange("(c p) f -> p c f", p=128))
            nc.gpsimd.dma_start(out=wv_sb, in_=wv.rearrange("(c p) f -> p c f", p=128))
            nc.gpsimd.dma_start(out=wo_sb, in_=wo.rearrange("(c p) j -> p c j", p=128))
            nc.gpsimd.dma_start(out=bo_sb, in_=bo)

            qT_sb = store.tile([128, fc, s], BF16, tag="qT")
            kT_sb = store.tile([128, fc, s], BF16, tag="kT")
            v_sb = store.tile([128, kc, hpc * 65], BF16, tag="v")
            outT_sb = store.tile([128, fc, s], BF16, tag="outT")

            # x slices are loaded once and shared between the prefix (feature
            # chunk 0) and the deferred per-chunk projections; the pool slot
            # stays live until the last emitted consumer
            xcache = {}

            def get_x(x_dram, q):
                key = (x_dram.tensor.name, q)
                if key not in xcache:
                    xr = x_dram.rearrange("(c p) s -> p c s", p=128)
                    t = xpool.tile([128, dc, aq], BF16, tag="x")
                    h2 = dc // 2
                    nc.sync.dma_start(out=t[:, :h2],
                                      in_=xr[:, :h2, q * aq:(q + 1) * aq])
                    nc.sync.dma_start(out=t[:, h2:],
                                      in_=xr[:, h2:, q * aq:(q + 1) * aq])
                    xcache[key] = t
                return xcache[key]

            def qk_proj_slice(x_dram, w_sb, dst, q, fis):
                """qT/kT projection for one aq slice and a list of f chunks."""
                sl = slice(q * aq, (q + 1) * aq)
                x_sb = get_x(x_dram, q)
                for fi in fis:
                    ps = psum.tile([128, aq], F32, tag="wide")
                    for qq in range(aq // 512):
                        for ci in range(dc):
                            nc.tensor.matmul(
                                ps[:, qq * 512:(qq + 1) * 512],
                                lhsT=w_sb[:, ci, fi * 128:(fi + 1) * 128],
                                rhs=x_sb[:, ci, qq * 512:(qq + 1) * 512],
                                start=(ci == 0), stop=(ci == dc - 1))
                    nc.vector.tensor_copy(out=dst[:, fi, sl], in_=ps)

            def v_proj_slice(q):
                """v projection (with ones columns interleaved) for one slice."""
                xr = xv.rearrange("(c p) s -> p c s", p=128)
                nchunks = aq // 128   # kj chunks per aq slice
                cpt = aq // f         # kj chunks packed per psum tile
                sl = slice(q * aq, (q + 1) * aq)
                x_sb = xpool.tile([128, dc, aq], BF16, tag="x")
                nc.sync.dma_start(out=x_sb, in_=xr[:, :, sl])
                for t in range(nchunks // cpt):
                    ps = psum.tile([128, aq], F32, tag="wide")
                    for k4 in range(cpt):
                        ck = t * cpt + k4
                        for ci in range(dc):
                            nc.tensor.matmul(
                                ps[:, k4 * f:(k4 + 1) * f],
                                lhsT=x_sb[:, ci, ck * 128:(ck + 1) * 128],
                                rhs=wv_sb[:, ci],
                                start=(ci == 0), stop=(ci == dc - 1))
                        cix = q * nchunks + ck
                        nc.vector.tensor_copy(
                            out=v_sb[:, cix].rearrange(
                                "p (h x) -> p h x", x=65)[:, :, 0:64],
                            in_=ps[:, k4 * f:(k4 + 1) * f].rearrange(
                                "p (h x) -> p h x", x=64))

            def attention(h, qb, queue=()):
                hp = (h % 2) * 64
                hc = h // 2
                kTh = kT_sb[hp:hp + 64, hc]
                qTh = qT_sb[hp:hp + 64, hc]
                oaug = psumo.tile([65, aq], F32, tag="oaug")
                for c in range(kc):
                    if queue and c in (kc // 3, (2 * kc) // 3):
                        queue.pop(0)()
                    sc = psum.tile([128, aq], F32, tag="wide")
                    for qq in range(aq // 512):
                        nc.tensor.matmul(
                            sc[:, qq * 512:(qq + 1) * 512],
                            lhsT=kTh[:, c * 128:(c + 1) * 128],
                            rhs=qTh[:, qb * aq + qq * 512:
                                    qb * aq + (qq + 1) * 512],
                            start=True, stop=True)
                    pr = ppool.tile([128, aq], BF16, tag="pr")
                    nc.scalar.activation(
                        out=pr, in_=sc,
                        func=mybir.ActivationFunctionType.Exp,
                        scale=float(scale))
                    for qq in range(aq // 512):
                        nc.tensor.matmul(
                            oaug[:, qq * 512:(qq + 1) * 512],
                            lhsT=v_sb[:, c, h * 65:(h + 1) * 65],
                            rhs=pr[:, qq * 512:(qq + 1) * 512],
                            start=(c == 0), stop=(c == kc - 1),
                            skip_group_check=True)
                # stage the denominator row to SBUF (DMA cannot read PSUM),
                # broadcast it over 64 partitions via a DRAM bounce, divide
                ls = bpool.tile([65, aq], F32, tag="ls")
                nc.vector.tensor_copy(out=ls, in_=oaug)
                ld = dpool.tile([1, aq], F32, tag="ld")
                nc.sync.dma_start(out=ld, in_=ls[64:65])
                bc = bpool.tile([64, aq], F32, tag="bc")
                nc.sync.dma_start(out=bc, in_=ld.to_broadcast([64, aq]))
                nc.vector.reciprocal_approx_fast(out=bc, in_=bc)
                od = opool.tile([64, aq], BF16, tag="od")
                nc.vector.tensor_mul(out=od, in0=ls[0:64], in1=bc)
                nc.sync.dma_start(
                    out=outT_sb[hp:hp + 64, hc, qb * aq:(qb + 1) * aq],
                    in_=od)

            def wo_proj_group(q, j):
                """output projection yT[j chunk, aq slice] = woT^T @ outT + bias"""
                sl = slice(q * aq, (q + 1) * aq)
                ps = psum.tile([128, aq], F32, tag="wide")
                for qq in range(aq // 512):
                    for fi in range(fc):
                        nc.tensor.matmul(
                            ps[:, qq * 512:(qq + 1) * 512],
                            lhsT=wo_sb[:, fi, j * 128:(j + 1) * 128],
                            rhs=outT_sb[:, fi,
                                        q * aq + qq * 512:q * aq + (qq + 1) * 512],
                            start=(fi == 0), stop=(fi == fc - 1))
                ys = ypool.tile([128, aq], F32, tag="ys")
                nc.scalar.activation(
                    out=ys, in_=ps,
                    func=mybir.ActivationFunctionType.Identity,
                    bias=bo_sb[:, j:j + 1], scale=1.0)
                nc.sync.dma_start(out=y[j * 128:(j + 1) * 128, sl], in_=ys)

            def body(_iv=None):
                # ones columns of v_aug (one strided memset per kj chunk)
                for c in range(kc):
                    nc.vector.memset(
                        v_sb[:, c].rearrange("p (h x) -> p h x", x=65)[:, :, 64:65],
                        1.0)

                # serial phases: projections, attention, output projection
                for q in range(nq):
                    qk_proj_slice(xk, wk_sb, kT_sb, q, range(fc))
                for q in range(nq):
                    qk_proj_slice(xq, wq_sb, qT_sb, q, range(fc))
                for q in range(nq):
                    v_proj_slice(q)
                for h in range(hpc):
                    for qb in range(nq):
                        attention(h, qb)
                for q in range(nq):
                    for j in range(jc):
                        wo_proj_group(q, j)

            if repeat == 1:
                body()
            else:
                with tc.For_i(0, repeat, 1) as iv:
                    body(iv)

    nc.compile()
    return nc


def make_core_inputs(query, key, value, wq, wk, wv, wo, bo):
    """Host-side sharding: per-core input dicts (bf16 casts + transposes)."""
    bf = ml_dtypes.bfloat16
    query, key, value = (np.asarray(t, np.float32) for t in (query, key, value))
    wq, wk, wv, wo, bo = (np.asarray(t, np.float32) for t in (wq, wk, wv, wo, bo))
    ins = []
    for c in range(N_CORES):
        b, g = c // 2, c % 2
        fs = slice(g * F, (g + 1) * F)
        ins.append({
            "xq_t": np.ascontiguousarray(query[b].astype(bf).T),
            "xk_t": np.ascontiguousarray(key[b].astype(bf).T),
            "xv_t": np.ascontiguousarray(value[b].astype(bf).T),
            "wq_t": np.ascontiguousarray(wq[fs, :].T.astype(bf)),
            "wk_t": np.ascontiguousarray(wk[fs, :].T.astype(bf)),
            "wv_t": np.ascontiguousarray(wv[fs, :].T.astype(bf)),
            "wo_t": np.ascontiguousarray(wo[:, fs].T.astype(bf)),
            "bo_r": (bo.reshape(D // 128, 128).T.astype(np.float32)
                     if g == 0 else np.zeros((128, D // 128), np.float32)),
        })
    return ins


_NC_CACHE = None


def kernel(query, key, value, wq, wk, wv, wo, bo):
    global _NC_CACHE
    from concourse.bass_utils import run_bass_kernel_spmd

    if _NC_CACHE is None:
        _NC_CACHE = build_nc()
    ins = make_core_inputs(query, key, value, wq, wk, wv, wo, bo)
    res = run_bass_kernel_spmd(_NC_CACHE, ins, list(range(N_CORES)))
    out = np.empty((B, S, D), np.float32)
    for b in range(B):
        out[b] = (res.results[2 * b]["y_t"] + res.results[2 * b + 1]["y_t"]).T
    return out
